# revision 1
# baseline (speedup 1.0000x reference)
"""Trainium2 Bass kernel for nn_AttentionPointnet (gnn_message_passing).

Data-parallel over batch: 8 NeuronCores x 1 sample each (B=8, T=4096).
Per-core program (v2 — restructured for engine balance):
  - KNN: m = 2 p.p_s - |p_s|^2 on PE in fp32 (512-wide chunks, exact so the
    top-20 selection matches the fp32 reference); per-row top-20 via
    per-128-chunk max8 -> merge rounds for the rank-20 threshold ->
    threshold mask -> exclusive-prefix ranks via bf16 strict-lower-
    triangular matmuls on PE -> GPSIMD local_scatter compacts the global
    indices (s-ascending order; softmax is order-invariant). Phase A runs
    in two 16-tile halves so gathers interleave with the DVE scan wall.
    The wrapped int16 gather-index image is built with ONE 16-partition
    DMA read + a PE replication matmul (not 8 element-scattered reads).
  - Scores use softmax shift-invariance: the per-query terms (p_t . wc[4:7]
    + bc) cancel, so score_eff[t,k] = wc0*dis[t,k] + G[idx[t,k]] where
    G[s,i] = p_s . wc_i[1:4] is host-precomputed into the gather payload.
    Softmax runs as 5 block-wide sweeps over [128, 640] (all 32 tiles at
    once), no max-subtraction needed (tiny score range).
  - Weighted K-sum on PE: per tile one DVE op builds a stacked diagonal
    diag(w_k) (ident-replica * w broadcast, bf16 2x mode), then 20 bf16
    matmuls accumulate att_raw^T = sum_k pooled_k^T diag(w_k) in PSUM.
  - Resnet/attention matmuls bf16 (1 cyc/row, 512-wide groups); shortcut
    and final projection read a per-group bf16 cast of the f32 net.
    Residual kept exact in f32 via DVE add from PSUM. Biases: b0 via ACT
    Relu bias; b1/bo/b_c as rank-1 bf16 matmuls.
  - Row-major bf16 net rebuilt per block via PE transposes into SBUF, then
    ONE 1MB DMA to DRAM; gathers use the production dma_gather path.
  - Output written feature-major [CDIM, T]; host transposes.
"""

import sys

for _p in ("/opt/trn_rl_repo", "/root/.axon_site/_ro/trn_rl_repo"):
    if _p not in sys.path:
        sys.path.append(_p)

import numpy as np
import ml_dtypes

import concourse.bass as bass
import concourse.bacc as bacc
import concourse.mybir as mybir
import concourse.tile as tile
from concourse import library_config

F32 = mybir.dt.float32
F32R = mybir.dt.float32r
U32 = mybir.dt.uint32
U16 = mybir.dt.uint16
I16 = mybir.dt.int16
BF16 = mybir.dt.bfloat16
AF = mybir.ActivationFunctionType
ALU = mybir.AluOpType
AX = mybir.AxisListType

B, T, D, H, NB, K, CDIM = 8, 4096, 3, 128, 6, 20, 128
NT = T // 128  # 32 t-tiles
NCHUNK = 32  # 128-wide chunks per row for stage-1 max8


def build_program():
    nc = bacc.Bacc("TRN2", target_bir_lowering=False, debug=False)

    # ---- DRAM I/O ----
    d_lhsT4 = nc.dram_tensor("lhsT4", [4, T], F32, kind="ExternalInput")
    d_rhs4 = nc.dram_tensor("rhs4", [4, T], F32, kind="ExternalInput")
    d_p = nc.dram_tensor("pdram", [T, 64], F32, kind="ExternalInput")
    d_wpos4 = nc.dram_tensor("wpos4", [4, H], F32, kind="ExternalInput")
    d_wc0 = nc.dram_tensor("wc0c", [128, NB], F32, kind="ExternalInput")
    d_b0col = nc.dram_tensor("b0col", [128, NB], F32, kind="ExternalInput")
    d_w0a = nc.dram_tensor("w0a", [NB, H, H], BF16, kind="ExternalInput")
    d_w0b = nc.dram_tensor("w0b", [NB, H, H], BF16, kind="ExternalInput")
    d_w1 = nc.dram_tensor("w1", [NB, H, H], BF16, kind="ExternalInput")
    d_wsa = nc.dram_tensor("wsa", [NB, H, H], BF16, kind="ExternalInput")
    d_wsb = nc.dram_tensor("wsb", [NB, H, H], BF16, kind="ExternalInput")
    d_wo = nc.dram_tensor("wo", [NB, H, H], BF16, kind="ExternalInput")
    d_wcf = nc.dram_tensor("wcfbf", [H, CDIM], BF16, kind="ExternalInput")
    d_ident = nc.dram_tensor("ident", [128, 128], F32, kind="ExternalInput")
    d_identbf = nc.dram_tensor("identbf", [128, 128], BF16, kind="ExternalInput")
    d_idrep = nc.dram_tensor("identrep", [128, 128 * K], BF16, kind="ExternalInput")
    d_ltbf = nc.dram_tensor("ltbf", [128, 128], BF16, kind="ExternalInput")
    d_onesmbf = nc.dram_tensor("onesmbf", [128, 128], BF16, kind="ExternalInput")
    d_browbf = nc.dram_tensor("browbf", [1, 18 * 128], BF16, kind="ExternalInput")
    d_coff = nc.dram_tensor("chunkoff", [128, 256], U16, kind="ExternalInput")
    d_repm = nc.dram_tensor("repmat", [16, 128], F32, kind="ExternalInput")
    d_ptile = nc.dram_tensor("ptile", [128, 3 * NT], F32, kind="ExternalInput")
    d_out = nc.dram_tensor("outp", [CDIM, T], F32, kind="ExternalOutput")

    from contextlib import ExitStack

    with tile.TileContext(nc) as tc:
        with ExitStack() as stk:
            constp = stk.enter_context(tc.tile_pool(name="const", bufs=1))
            pers = stk.enter_context(tc.tile_pool(name="pers", bufs=1))
            dramp = stk.enter_context(tc.tile_pool(name="dram", bufs=2, space="DRAM"))
            gdramp = stk.enter_context(tc.tile_pool(name="gdram", bufs=NT, space="DRAM"))
            smallp = stk.enter_context(tc.tile_pool(name="small", bufs=3))
            pgp = stk.enter_context(tc.tile_pool(name="pgp", bufs=2))
            pooledp = stk.enter_context(tc.tile_pool(name="pooled", bufs=4))
            dstkp = stk.enter_context(tc.tile_pool(name="dstk", bufs=1))
            gbufp = stk.enter_context(tc.tile_pool(name="gbuf", bufs=2))
            sbw = stk.enter_context(tc.tile_pool(name="sbw", bufs=6))
            outbp = stk.enter_context(tc.tile_pool(name="outb", bufs=1))
            swp = stk.enter_context(tc.tile_pool(name="swp", bufs=1))
            psf = stk.enter_context(tc.tile_pool(name="psf", bufs=2, space="PSUM"))
            nc.gpsimd.load_library(library_config.local_scatter)
            # ---- load constants into SBUF ----
            wpos4 = constp.tile([4, H], F32, tag="wpos4")
            wc0c = constp.tile([128, NB], F32, tag="wc0c")
            b0col = constp.tile([128, NB], F32, tag="b0col")
            ident = constp.tile([128, 128], F32, tag="ident")
            idrep = constp.tile([128, 128 * K], BF16, tag="idrep")
            ptile = constp.tile([128, 3 * NT], F32, tag="ptile")
            nc.sync.dma_start(ptile[:, :], d_ptile.ap())
            browbf = constp.tile([1, 18 * 128], BF16, tag="browbf")
            onesbf = constp.tile([1, 512], BF16, tag="onesbf")

            w0a = constp.tile([128, NB, H], BF16, tag="w0a")
            w0b = constp.tile([128, NB, H], BF16, tag="w0b")
            w1 = constp.tile([128, NB, H], BF16, tag="w1")
            wsa = constp.tile([128, NB, H], BF16, tag="wsa")
            wsb = constp.tile([128, NB, H], BF16, tag="wsb")
            wo = constp.tile([128, NB, H], BF16, tag="wo")
            wcfbf = constp.tile([128, CDIM], BF16, tag="wcfbf")

            nc.sync.dma_start(wpos4[:, :], d_wpos4.ap())
            nc.sync.dma_start(wc0c[:, :], d_wc0.ap())
            nc.sync.dma_start(b0col[:, :], d_b0col.ap())
            nc.sync.dma_start(ident[:, :], d_ident.ap())
            nc.sync.dma_start(idrep[:, :], d_idrep.ap())
            nc.sync.dma_start(browbf[:, :], d_browbf.ap())
            nc.sync.dma_start(onesbf[:, :], d_browbf.ap()[0:1, 13 * 128:13 * 128 + 512])
            nc.sync.dma_start(wcfbf[:, :], d_wcf.ap())
            for sb_t, dr in (
                (w0a, d_w0a), (w0b, d_w0b), (w1, d_w1),
                (wsa, d_wsa), (wsb, d_wsb), (wo, d_wo),
            ):
                nc.sync.dma_start(
                    sb_t[:, :, :], dr.ap().rearrange("i hin hout -> hin i hout")
                )

            # ---- persistent activations ----
            netA = pers.tile([128, T], F32, tag="netA")  # feature-major net
            netB = pers.tile([128, T], F32, tag="netB")
            dis = pers.tile([128, NT * K], F32, tag="dis")  # distances
            Gp = pers.tile([128, NT * K * NB], BF16, tag="Gp")  # G[idx] per blk
            idx16 = pers.tile([128, NT * 160], I16, tag="idx16")
            w20all = pers.tile([128, NB * NT * K], BF16, tag="w20all")

            # ================= Phase A: net0 + KNN =================
            stkA = ExitStack()
            constA = stkA.enter_context(tc.tile_pool(name="constA", bufs=1))
            marrp = stkA.enter_context(tc.tile_pool(name="marr", bufs=2))
            ltp = stkA.enter_context(tc.tile_pool(name="ltp", bufs=2))
            ltp1 = stkA.enter_context(tc.tile_pool(name="ltp1", bufs=1))
            psmarr = stkA.enter_context(tc.tile_pool(name="psmarr", bufs=3, space="PSUM"))
            psbf = stkA.enter_context(tc.tile_pool(name="psbf", bufs=1, space="PSUM"))
            rhs4 = constA.tile([4, T], F32, tag="rhs4")
            nc.sync.dma_start(rhs4[:, :], d_rhs4.ap())
            identbf = constA.tile([128, 128], BF16, tag="identbf")
            ltbf = constA.tile([128, 128], BF16, tag="ltbf")
            onesmbf = constA.tile([128, 128], BF16, tag="onesmbf")
            coff = constA.tile([128, 256], U16, tag="coff")
            repm = constA.tile([16, 128], F32, tag="repm")
            nc.sync.dma_start(repm[:, :], d_repm.ap())
            stkA_ptile = None
            nc.sync.dma_start(identbf[:, :], d_identbf.ap())
            nc.sync.dma_start(ltbf[:, :], d_ltbf.ap())
            nc.sync.dma_start(onesmbf[:, :], d_onesmbf.ap())
            nc.sync.dma_start(coff[:, :], d_coff.ap())
            # net0 feature-major: netA = wpos4.T @ [p;1]  (f32r, 512-wide)
            for c in range(8):
                p1c = ltp1.tile([4, 512], F32, tag="p1c")
                nc.sync.dma_start(p1c[:, :], d_lhsT4.ap()[:, c * 512:(c + 1) * 512])
                psn = psmarr.tile([128, 512], F32, tag="psmarr")
                nc.tensor.matmul(psn[:, :], lhsT=wpos4[:, :],
                                 rhs=p1c[:, :], start=True, stop=True)
                nc.scalar.copy(netA[:, c * 512:(c + 1) * 512], psn[:, :])
            # block-0 row-major net to DRAM early (overlaps the KNN phase)
            netdram0 = dramp.tile([T, H], BF16, tag="netdram")
            for j in range(NT):
                psT = psf.tile([128, 128], F32, tag="psf")
                nc.tensor.transpose(psT[:, :], netA[:, j * 128:(j + 1) * 128],
                                    ident[:, :])
                nrow = sbw.tile([128, 128], BF16, tag="nrow")
                nc.scalar.copy(nrow[:, :], psT[:, :])
                nc.sync.dma_start(netdram0[j * 128:(j + 1) * 128, :], nrow[:, :])

            HALF = NT // 2
            gdram_tiles = [None] * NT
            pend = []  # p-gathers awaiting consumption (j, pg)

            def consume_pg(jj, pgt):
                # G values for all 6 blocks (bf16)
                gslice = Gp[:, jj * K * NB:(jj + 1) * K * NB]
                nc.vector.tensor_copy(
                    gslice.rearrange("p (k i) -> p k i", i=NB),
                    pgt[:, :, 3:3 + NB],
                )
                # d2 = |p_t - p_s|^2 from gathered rows (sqrt batched later)
                d2t = smallp.tile([128, K], F32, tag="d2t")
                df = smallp.tile([128, K], F32, tag="df")
                for d in range(D):
                    nc.vector.tensor_scalar(
                        df[:, :], pgt[:, :, d], -1.0,
                        ptile[:, 3 * jj + d:3 * jj + d + 1],
                        op0=ALU.mult, op1=ALU.add,
                    )
                    if d == 0:
                        nc.vector.tensor_tensor(d2t[:, :], df[:, :], df[:, :],
                                                op=ALU.mult)
                    else:
                        nc.vector.tensor_tensor(df[:, :], df[:, :], df[:, :],
                                                op=ALU.mult)
                        nc.vector.tensor_tensor(d2t[:, :], d2t[:, :], df[:, :],
                                                op=ALU.add)
                nc.vector.tensor_scalar_max(dis[:, jj * K:(jj + 1) * K],
                                            d2t[:, :], 1e-12)

            for h in range(2):
              tiles = list(range(h * HALF, (h + 1) * HALF))
              if h > 0:
                nc.gpsimd.load_library(library_config.local_scatter)
              for idx_h, j in enumerate(tiles):
                ltt = ltp.tile([4, 128], F32, tag="ltt")
                nc.sync.dma_start(ltt[:, :], d_lhsT4.ap()[:, j * 128:(j + 1) * 128])
                lt = ltt[:, :]
                marr = marrp.tile([128, T], F32, tag="marr")
                for c in range(8):
                    ps = psmarr.tile([128, 512], F32, tag="psmarr")
                    nc.tensor.matmul(ps[:, :], lhsT=lt,
                                     rhs=rhs4[:, c * 512:(c + 1) * 512],
                                     start=True, stop=True)
                    nc.scalar.copy(marr[:, c * 512:(c + 1) * 512], ps[:, :])

                # ---- stage-1 top-8 per 128-chunk ----
                cand = smallp.tile([128, 256], F32, tag="cand")
                for c in range(NCHUNK):
                    nc.vector.max(cand[:, c * 8:(c + 1) * 8],
                                  marr[:, c * 128:(c + 1) * 128])
                vals = smallp.tile([128, 24], F32, tag="vals")
                wk1 = smallp.tile([128, 256], F32, tag="wk1")
                nc.vector.max(vals[:, 0:8], cand[:, :])
                nc.vector.match_replace(wk1[:, :], vals[:, 0:8], cand[:, :], -1e30)
                nc.vector.max(vals[:, 8:16], wk1[:, :])
                nc.vector.match_replace(wk1[:, :], vals[:, 8:16], wk1[:, :], -1e30)
                nc.vector.max(vals[:, 16:24], wk1[:, :])


                # per-chunk indices -> global candidate index table
                lidx = smallp.tile([128, 256], U16, tag="lidx")
                for c in range(NCHUNK):
                    nc.vector.max_index(lidx[:, c * 8:(c + 1) * 8],
                                        cand[:, c * 8:(c + 1) * 8],
                                        marr[:, c * 128:(c + 1) * 128])
                nc.vector.tensor_tensor(lidx[:, :], lidx[:, :], coff[:, :],
                                        op=ALU.add)

                # selection mask O = cand >= tau (tau = 20th largest), bf16
                Om = smallp.tile([128, 256], BF16, tag="Om")
                nc.vector.tensor_scalar(Om[:, :], cand[:, :], vals[:, 19:20],
                                        None, op0=ALU.is_ge)
                # exclusive prefix ranks via PE (bf16; counts <= 256 exact)
                psT0 = psbf.tile([128, 128], BF16, tag="psbf")
                nc.tensor.transpose(psT0[:, :], Om[:, 0:128], identbf[:, :])
                ot0 = smallp.tile([128, 128], BF16, tag="ot0")
                nc.scalar.copy(ot0[:, :], psT0[:, :])
                psT1 = psbf.tile([128, 128], BF16, tag="psbf")
                nc.tensor.transpose(psT1[:, :], Om[:, 128:256], identbf[:, :])
                ot1 = smallp.tile([128, 128], BF16, tag="ot1")
                nc.scalar.copy(ot1[:, :], psT1[:, :])
                psP0 = psf.tile([128, 128], F32, tag="psf")
                nc.tensor.matmul(psP0[:, :], lhsT=ltbf[:, :], rhs=ot0[:, :],
                                 start=True, stop=True)
                pf0 = smallp.tile([128, 128], BF16, tag="pf0")
                nc.scalar.copy(pf0[:, :], psP0[:, :])
                psP1 = psf.tile([128, 128], F32, tag="psf")
                nc.tensor.matmul(psP1[:, :], lhsT=onesmbf[:, :], rhs=ot0[:, :],
                                 start=True, stop=False)
                nc.tensor.matmul(psP1[:, :], lhsT=ltbf[:, :], rhs=ot1[:, :],
                                 start=False, stop=True)
                pf1 = smallp.tile([128, 128], BF16, tag="pf1")
                nc.scalar.copy(pf1[:, :], psP1[:, :])
                # back-transpose prefix to [t, s]
                pfx = smallp.tile([128, 256], BF16, tag="pfx")
                psB0 = psbf.tile([128, 128], BF16, tag="psbf")
                nc.tensor.transpose(psB0[:, :], pf0[:, :], identbf[:, :])
                nc.scalar.copy(pfx[:, 0:128], psB0[:, :])
                psB1 = psbf.tile([128, 128], BF16, tag="psbf")
                nc.tensor.transpose(psB1[:, :], pf1[:, :], identbf[:, :])
                nc.scalar.copy(pfx[:, 128:256], psB1[:, :])
                # sidx = O ? rank : -1   (as int16)
                om1 = smallp.tile([128, 256], BF16, tag="om1")
                nc.vector.tensor_scalar(om1[:, :], Om[:, :], -1.0, None,
                                        op0=ALU.add)
                nc.vector.tensor_tensor(pfx[:, :], pfx[:, :], Om[:, :],
                                        op=ALU.mult)
                sidx = smallp.tile([128, 256], I16, tag="sidx")
                nc.vector.tensor_tensor(sidx[:, :], pfx[:, :], om1[:, :],
                                        op=ALU.add)
                # compact indices to 32 slots (ranks 0..19 used)
                gidx24 = smallp.tile([128, 32], I16, tag="gidx24")
                nc.gpsimd.local_scatter(gidx24[:, :], lidx[:, :], sidx[:, :],
                                        channels=128, num_elems=32, num_idxs=256)
                gdram = gdramp.tile([128, K], I16, tag="gdram")
                nc.sync.dma_start(gdram[:, :], gidx24[:, 0:K])
                gdram_tiles[j] = gdram
                srca = bass.AP(
                    tensor=gdram[:, :].tensor,
                    offset=gdram[:, :].offset,
                    ap=[[K, 16], [1, K], [16 * K, 8]],
                )
                ixq = smallp.tile([16, 160], I16, tag="ixq")
                nc.sync.dma_start(
                    ixq[:, :].rearrange("q (k g) -> q k g", k=K), srca)
                ixqf = smallp.tile([16, 160], F32, tag="ixqf")
                nc.scalar.copy(ixqf[:, :], ixq[:, :])
                psI = psbf.tile([128, 160], F32, tag="psidx")
                nc.tensor.matmul(psI[:, :], lhsT=repm[:, :], rhs=ixqf[:, :],
                                 start=True, stop=True)
                nc.vector.tensor_copy(idx16[:, j * 160:(j + 1) * 160], psI[:, :])
                # interleave previous half's p-gather consumption under this wall
                if idx_h < len(pend):
                    consume_pg(*pend[idx_h])

              if pend:
                # batched in-place sqrt for the previous half's distances
                ph = slice((h - 1) * HALF * K, h * HALF * K)
                nc.scalar.activation(dis[:, ph], dis[:, ph], AF.Sqrt)
              # switch GPSIMD library, issue this half's p-row gathers
              nc.gpsimd.load_library(library_config.mlp)
              pend = []
              for j in tiles:
                pg = pgp.tile([128, K, 64], F32, tag="pg")
                nc.gpsimd.dma_gather(
                    out_ap=pg[:, :, :], in_ap=d_p.ap(),
                    idxs_ap=idx16[:, j * 160:(j + 1) * 160],
                    num_idxs=128 * K, num_idxs_reg=128 * K, elem_size=64,
                    single_packet=False,
                )
                pend.append((j, pg))

            for _pj, _pg in pend:
                consume_pg(_pj, _pg)
            pend = []
            nc.scalar.activation(dis[:, HALF * K:], dis[:, HALF * K:], AF.Sqrt)

            # ---- softmax sweeps: all tiles, after both halves ----
            Gp4 = Gp[:, :].rearrange("p (j k i) -> p j k i", k=K, i=NB)
            for h in range(2):
              hs = slice(h * HALF * K, (h + 1) * HALF * K)
              for i in range(NB):
                tmp = swp.tile([128, HALF * K], F32, tag="swtmp")
                nc.vector.tensor_scalar(tmp[:, :], dis[:, hs], wc0c[:, i:i + 1],
                                        None, op0=ALU.mult)
                tmp3 = tmp[:, :].rearrange("p (j k) -> p j k", k=K)
                nc.vector.tensor_tensor(
                    tmp3, tmp3, Gp4[:, h * HALF:(h + 1) * HALF, :, i],
                    op=ALU.add)
                esc = swp.tile([128, HALF * K], BF16, tag="esc")
                nc.scalar.activation(esc[:, :], tmp[:, :], AF.Exp)
                esc3 = esc[:, :].rearrange("p (j k) -> p j k", k=K)
                den = smallp.tile([128, HALF], F32, tag="den")
                nc.vector.tensor_reduce(den[:, :], esc3, axis=AX.X, op=ALU.add)
                rden = smallp.tile([128, HALF], F32, tag="rden")
                nc.vector.reciprocal(rden[:, :], den[:, :])
                rbase = rden[:, :]
                rbcast = bass.AP(tensor=rbase.tensor, offset=rbase.offset,
                                 ap=[rbase.ap[0], [1, HALF], [0, K]])
                w20s = w20all[:, i * NT * K + h * HALF * K:
                              i * NT * K + (h + 1) * HALF * K].rearrange(
                    "p (j k) -> p j k", k=K)
                nc.vector.tensor_tensor(w20s, esc3, rbcast, op=ALU.mult)

            stkA.close()
            stkB = ExitStack()
            psbig = stkB.enter_context(tc.tile_pool(name="psbig", bufs=3, space="PSUM"))
            psatt = stkB.enter_context(tc.tile_pool(name="psatt", bufs=3, space="PSUM"))

            # ================= Phase B: blocks =================
            idrep3 = idrep[:, :].rearrange("p (s k) -> p s k", k=K)
            netdram = netdram0
            for i in range(NB):
                nin = netA if i % 2 == 0 else netB
                nout = netB if i % 2 == 0 else netA
                if i < NB - 1:
                    netdram_next = dramp.tile([T, H], BF16, tag="netdram")
                else:
                    netdram_next = None

                for g in range(8):
                    attT4 = gbufp.tile([128, 512], BF16, tag="attT4")
                    reluA4 = gbufp.tile([128, 512], BF16, tag="reluA4")
                    for jj in range(4):
                        j = 4 * g + jj
                        if True:
                            pooled = pooledp.tile([128, K, H], BF16, tag="pooled")
                            nc.gpsimd.dma_gather(
                                out_ap=pooled[:, :, :], in_ap=netdram[:, :],
                                idxs_ap=idx16[:, j * 160:(j + 1) * 160],
                                num_idxs=128 * K, num_idxs_reg=128 * K,
                                elem_size=H, single_packet=False,
                            )
                            # stacked diagonals: dstack[t, s, k] = w20[t,k]*(s==t)
                            dstack = dstkp.tile([128, 128 * K], BF16, tag="dstack")
                            dstack3 = dstack[:, :].rearrange("p (s k) -> p s k", k=K)
                            wbase = w20all[:, (i * NT + j) * K:(i * NT + j + 1) * K]
                            wb = bass.AP(tensor=wbase.tensor, offset=wbase.offset,
                                         ap=[wbase.ap[0], [0, 128], [1, K]])
                            nc.vector.tensor_tensor(dstack3, idrep3, wb,
                                                    op=ALU.mult)
                            # att_raw^T = sum_k pooled_k^T diag(w_k) on PE
                            psR = psatt.tile([128, 128], F32, tag="psatt")
                            for k in range(K):
                                nc.tensor.matmul(
                                    psR[:, :],
                                    lhsT=pooled[:, k, :],
                                    rhs=dstack3[:, :, k],
                                    start=(k == 0), stop=(k == K - 1),
                                )
                            attrawT = sbw.tile([128, 128], BF16, tag="attrawT")
                            nc.scalar.copy(attrawT[:, :], psR[:, :])
                            psA = psatt.tile([128, 128], F32, tag="psatt")
                            nc.tensor.matmul(psA[:, :], lhsT=wo[:, i, :],
                                             rhs=attrawT[:, :],
                                             start=True, stop=False)
                            nc.tensor.matmul(
                                psA[:, :],
                                lhsT=browbf[:, (6 + i) * 128:(7 + i) * 128],
                                rhs=onesbf[:, 0:128], start=False, stop=True)
                            nc.scalar.activation(
                                reluA4[:, jj * 128:(jj + 1) * 128], psA[:, :],
                                AF.Relu)
                            nc.vector.tensor_copy(
                                attT4[:, jj * 128:(jj + 1) * 128], psA[:, :])

                    # ---- group (512-wide) resnet matmuls ----
                    nsl = slice(g * 512, (g + 1) * 512)
                    reluN = gbufp.tile([128, 512], BF16, tag="reluN")
                    nc.scalar.activation(reluN[:, :], nin[:, nsl], AF.Relu)
                    netbf = gbufp.tile([128, 512], BF16, tag="netbf")
                    nc.scalar.copy(netbf[:, :], nin[:, nsl])
                    ps1 = psbig.tile([128, 512], F32, tag="psbig")
                    nc.tensor.matmul(ps1[:, :], lhsT=w0a[:, i, :],
                                     rhs=reluN[:, :], start=True, stop=False)
                    nc.tensor.matmul(ps1[:, :], lhsT=w0b[:, i, :],
                                     rhs=reluA4[:, :], start=False, stop=True)
                    hrelu = gbufp.tile([128, 512], BF16, tag="hrelu")
                    nc.scalar.activation(hrelu[:, :], ps1[:, :], AF.Relu,
                                         bias=b0col[:, i:i + 1], scale=1.0)
                    ps2 = psbig.tile([128, 512], F32, tag="psbig")
                    nc.tensor.matmul(ps2[:, :], lhsT=w1[:, i, :],
                                     rhs=hrelu[:, :], start=True, stop=False)
                    nc.tensor.matmul(ps2[:, :], lhsT=wsa[:, i, :],
                                     rhs=netbf[:, :], start=False, stop=False)
                    nc.tensor.matmul(ps2[:, :], lhsT=wsb[:, i, :],
                                     rhs=attT4[:, :], start=False, stop=False)
                    nc.tensor.matmul(ps2[:, :],
                                     lhsT=browbf[:, i * 128:(i + 1) * 128],
                                     rhs=onesbf[:, 0:512], start=False, stop=True)
                    if i == 0:
                        nc.vector.tensor_copy(nout[:, nsl], ps2[:, :])
                    else:
                        nc.vector.tensor_tensor(nout[:, nsl], ps2[:, :],
                                                nin[:, nsl], op=ALU.add)
                    # incrementally build next block's row-major net
                    if netdram_next is not None:
                        for jj in range(4):
                            j = 4 * g + jj
                            psT = psf.tile([128, 128], F32, tag="psf")
                            nc.tensor.transpose(
                                psT[:, :], nout[:, j * 128:(j + 1) * 128],
                                ident[:, :])
                            nrow = sbw.tile([128, 128], BF16, tag="nrow")
                            nc.scalar.copy(nrow[:, :], psT[:, :])
                            nc.sync.dma_start(
                                netdram_next[j * 128:(j + 1) * 128, :],
                                nrow[:, :])

                netdram = netdram_next

            # ================= Final projection =================
            net6 = netA  # after 6 blocks output is back in netA
            for g in range(8):
                nsl = slice(g * 512, (g + 1) * 512)
                net6bf = gbufp.tile([128, 512], BF16, tag="netbf")
                nc.scalar.copy(net6bf[:, :], net6[:, nsl])
                psF = psbig.tile([128, 512], F32, tag="psbig")
                nc.tensor.matmul(psF[:, :], lhsT=wcfbf[:, :],
                                 rhs=net6bf[:, :], start=True, stop=False)
                nc.tensor.matmul(psF[:, :],
                                 lhsT=browbf[:, 12 * 128:13 * 128],
                                 rhs=onesbf[:, 0:512], start=False, stop=True)
                og = outbp.tile([128, 512], F32, tag="og")
                nc.scalar.copy(og[:, :], psF[:, :])
                nc.sync.dma_start(d_out.ap()[:, nsl], og[:, :])
            stkB.close()

    nc.compile()
    return nc


def make_inputs(p_all, weights):
    """Build the per-core input maps. p_all: [B, T, D] f32."""
    w = weights
    bf = ml_dtypes.bfloat16
    shared = {}
    shared["wpos4"] = np.concatenate(
        [w["W_pos"], w["b_pos"][None, :]], axis=0
    ).astype(np.float32)  # [4, H]
    wc = np.asarray(w["att_Wc"][:, :, 0], np.float32)  # [NB, 7]
    shared["wc0c"] = np.broadcast_to(wc[:, 0][None, :], (128, NB)).astype(
        np.float32).copy()
    shared["b0col"] = np.ascontiguousarray(
        np.asarray(w["blk_b0"], np.float32).T)  # [H, NB]
    shared["w0a"] = np.ascontiguousarray(w["blk_W0"][:, :H, :]).astype(bf)
    shared["w0b"] = np.ascontiguousarray(w["blk_W0"][:, H:, :]).astype(bf)
    shared["w1"] = np.ascontiguousarray(w["blk_W1"]).astype(bf)
    shared["wsa"] = np.ascontiguousarray(w["blk_Ws"][:, :H, :]).astype(bf)
    shared["wsb"] = np.ascontiguousarray(w["blk_Ws"][:, H:, :]).astype(bf)
    shared["wo"] = np.ascontiguousarray(w["att_Wo"]).astype(bf)
    shared["wcfbf"] = np.ascontiguousarray(w["W_c"]).astype(bf)
    shared["ident"] = np.eye(128, dtype=np.float32)
    shared["identbf"] = np.eye(128, dtype=np.float32).astype(bf)
    idr = np.zeros((128, 128, K), np.float32)
    for t in range(128):
        idr[t, t, :] = 1.0
    shared["identrep"] = idr.reshape(128, 128 * K).astype(bf)
    brow = np.zeros((1, 18 * 128), np.float32)
    for i in range(NB):
        brow[0, i * 128:(i + 1) * 128] = w["blk_b1"][i]
        brow[0, (6 + i) * 128:(7 + i) * 128] = w["att_bo"][i]
    brow[0, 12 * 128:13 * 128] = w["b_c"]
    brow[0, 13 * 128:17 * 128] = 1.0
    shared["browbf"] = brow.astype(bf)
    shared["ltbf"] = np.triu(np.ones((128, 128), np.float32), 1).T.astype(bf)
    shared["onesmbf"] = np.ones((128, 128), np.float32).astype(bf)
    co = np.zeros((128, 256), np.uint16)
    co[:, :] = (np.arange(256) // 8 * 128)[None, :]
    shared["chunkoff"] = co
    rm = np.zeros((16, 128), np.float32)
    for p in range(128):
        rm[p % 16, p] = 1.0
    shared["repmat"] = rm

    wcg = wc[:, 1:4]  # [NB, 3]

    in_maps = []
    for c in range(B):
        p = np.asarray(p_all[c], np.float32)  # [T, D]
        sq = (p * p).sum(-1)  # [T]
        m = dict(shared)
        lhsT4 = np.ones((4, T), np.float32)
        lhsT4[0:3, :] = p.T
        m["lhsT4"] = lhsT4
        rhs4 = np.empty((4, T), np.float32)
        rhs4[0:3, :] = 2.0 * p.T
        rhs4[3, :] = -sq
        m["rhs4"] = rhs4
        pp = np.zeros((T, 64), np.float32)
        pp[:, :D] = p
        pp[:, 3:3 + NB] = p @ wcg.T  # G[s, i] = p_s . wc_i[1:4]
        m["pdram"] = pp
        pt = np.zeros((128, 3 * NT), np.float32)
        for j in range(NT):
            pt[:, 3 * j:3 * j + 3] = p[j * 128:(j + 1) * 128]
        m["ptile"] = pt
        in_maps.append(m)
    return in_maps


_PROGRAM = None


def kernel(**inputs):
    global _PROGRAM
    p_all = np.asarray(inputs["p"], np.float32)
    assert p_all.shape == (B, T, D)
    in_maps = make_inputs(p_all, {k: np.asarray(v) for k, v in inputs.items()})
    if _PROGRAM is None:
        _PROGRAM = build_program()
    from concourse import bass_utils
    res = bass_utils.run_bass_kernel_spmd(
        _PROGRAM, in_maps, core_ids=list(range(B))
    )
    out = np.stack([r["outp"].T for r in res.results], axis=0)  # [B, T, CDIM]
    return np.ascontiguousarray(out).astype(np.float32)


if __name__ == "__main__":
    # smoke: build only
    nc = build_program()
    print("built ok")

